# revision 6
# baseline (speedup 1.0000x reference)
"""KoLeo loss kernel for Trainium2 (8 NeuronCores, Bass/Tile), fp8 edition.

reference semantics:
    x = student_output / max(||row||_2, 1e-8)        # [B, D] row-normalize
    dots = x @ x.T ; dots[i,i] = -1
    nn = argmax(dots, axis=1)
    d_i = || x_i - x_nn(i) + 1e-8 ||_2
    loss = mean(-log(d_i + 1e-8))

Device strategy (data-parallel over rows, 8 cores, identical NEFF):
  * Host pre-normalizes rows in fp32, scales by S=128, quantizes to
    fp8 e4m3 (TRN FP8_EXP4: max normal 240 > S) and ships the
    transposed layout [KT=8, 128, B].  Numpy-validated rel err 1.5e-4.
  * Gram tiles run as fp8 DoubleRow matmuls: each MM consumes TWO
    128-deep k-tiles ([128, 2, free] APs) at 2x bf16 PE throughput.
    Rows are unit-norm pre-quantization so no scaling pass is needed;
    the diagonal entry ~S^2 dominates, the global 2nd-max of a row is
    S^2 * the NN cosine.
  * Input DMAs alternate between both HWDGE queues (SP / ACT) at
    k-tile granularity, ordered stripe-0-first so the first DoubleRow
    matmul is gated by ~4 slices, not the whole 8 MB load.
  * Loop: jh-stripes of 2048 columns outer, mt (128-row chunk) inner;
    per (stripe, mt) group 4 psum tiles [128, 512] each accumulate 4
    DoubleRow MMs; DVE max8 drains PSUM directly into a candidate
    buffer.  4 psum tags x 2 bufs = all 8 PSUM banks ping-pong; dummy
    matmuls during the first DMA keep the PE/HAM clock warm.
  * Device ships the top-2 per (mt, j-tile) candidate values; the host
    takes the global 2nd max per row (top-1 is the row's self-dot),
    computes d^2 = 2 - 2 m~ / S^2 and the final mean of logs.
"""

import numpy as np
import ml_dtypes

import concourse.bacc as bacc
import concourse.bass as bass
import concourse.mybir as mybir
import concourse.tile as tile
from concourse import bass_utils

B, D, P = 8192, 1024, 128
NCORES = 8
LOCAL = B // NCORES  # 1024 rows per core
KT = D // P          # 8 contraction tiles
MT = LOCAL // P      # 8 local row tiles
NJ = 512             # moving free dim per matmul
JT = B // NJ         # 16 column tiles
JH = 4               # j-tiles per stripe
NSTRIPE = JT // JH   # 4 stripes of 2048 columns
SCALE = 128.0        # fp8 pre-scale; diag ~ S^2

F32 = mybir.dt.float32
FP8 = mybir.dt.float8e4
DR = mybir.MatmulPerfMode.DoubleRow


def emit_kernel(tc, x_ap, xl_ap, out_ap):
    nc = tc.nc
    with (
        tc.tile_pool(name="big", bufs=1) as big,
        tc.tile_pool(name="ps", bufs=2, space="PSUM") as pp,
    ):
        xT = big.tile([P, KT, B], FP8)
        xTl = big.tile([P, KT, LOCAL], FP8)
        cand = big.tile([P, MT, JT, 8], F32)
        warm = big.tile([P, NJ], FP8)

        nc.vector.memset(warm[:], 1.0)

        # --- loads: host ships fp8 already transposed as [KT, 128, B].
        # k-granular, stripe-0 first, alternating both HWDGE queues so the
        # first matmul group is gated by ~4 slices instead of the full 8 MB.
        for k in range(KT):
            nc.sync.dma_start(out=xTl[:, k], in_=xl_ap[k])
            cb = slice(0, JH * NJ)
            nc.scalar.dma_start(out=xT[:, k, cb], in_=x_ap[k, :, cb])
        for s in range(1, NSTRIPE):
            cb = slice(s * JH * NJ, (s + 1) * JH * NJ)
            for k in range(KT):
                q = nc.sync if k % 2 == 0 else nc.scalar
                q.dma_start(out=xT[:, k, cb], in_=x_ap[k, :, cb])

        # --- PE/HAM pre-warm: dummy matmuls on the memset tile while the
        # first stripe DMA is in flight (borrows one main-pool psum buffer;
        # finishes long before its rotation comes around again).
        wps = pp.tile([P, NJ], F32, tag="ps_u0", name="wps")
        for _ in range(12):
            nc.tensor.matmul(wps[:], warm[:, :P], warm[:], start=True, stop=True)

        # --- Gram: fp8 DoubleRow, 2 k-tiles per MM ------------------------
        for s in range(NSTRIPE):
            for mt in range(MT):
                ms = slice(mt * P, (mt + 1) * P)
                pss = [
                    pp.tile([P, NJ], F32, tag=f"ps_u{jj}", name=f"ps_u{jj}")
                    for jj in range(JH)
                ]
                for kk in range(KT // 2):
                    ks = slice(2 * kk, 2 * kk + 2)
                    for jj in range(JH):
                        j = s * JH + jj
                        nc.tensor.matmul(
                            pss[jj][:],
                            xTl[:, ks, ms],
                            xT[:, ks, j * NJ : (j + 1) * NJ],
                            start=(kk == 0),
                            stop=(kk == KT // 2 - 1),
                            perf_mode=DR,
                        )
                for jj in range(JH):
                    nc.vector.max(out=cand[:, mt, s * JH + jj], in_=pss[jj][:])

        # --- ship top-2 per (mt, j-tile); host does 2nd-max + log ---------
        nc.sync.dma_start(out=out_ap, in_=cand[:, :, :, 0:2])


def build_bass():
    nc = bacc.Bacc(
        "TRN2",
        target_bir_lowering=False,
        debug=False,
        enable_asserts=True,
        num_devices=NCORES,
    )
    x_t = nc.dram_tensor("xq", [KT, P, B], FP8, kind="ExternalInput").ap()
    xl_t = nc.dram_tensor("xlq", [KT, P, LOCAL], FP8, kind="ExternalInput").ap()
    out_t = nc.dram_tensor("cand2", [P, MT, JT, 2], F32, kind="ExternalOutput").ap()
    with tile.TileContext(nc) as tc:
        emit_kernel(tc, x_t, xl_t, out_t)
    nc.compile()
    return nc


def make_in_maps(x: np.ndarray):
    norm = np.linalg.norm(x, axis=1, keepdims=True)
    xn = x / np.maximum(norm, 1e-8)
    q = (SCALE * xn).astype(ml_dtypes.float8_e4m3)
    # [KT, P, B]: element [k, p, r] = q[r, k*128 + p]  (transposed layout)
    xt = np.ascontiguousarray(q.reshape(B, KT, P).transpose(1, 2, 0))
    return [
        {
            "xq": xt,
            "xlq": np.ascontiguousarray(xt[:, :, c * LOCAL : (c + 1) * LOCAL]),
        }
        for c in range(NCORES)
    ]


def reduce_outputs(results):
    # cand2[p, mt, j, 0:2]: top-2 of S^2 * dots for local row mt*128+p over
    # column tile j.  Global top-1 is the self-dot (~S^2); the NN dot is the
    # global 2nd max.
    vals = np.concatenate(
        [r["cand2"].astype(np.float32).reshape(P, MT, JT * 2) for r in results],
        axis=1,
    )  # [P, NCORES*MT, 32]
    vals = vals.transpose(1, 0, 2).reshape(B, JT * 2)  # row-major [B, 32]
    part = np.partition(vals, JT * 2 - 2, axis=1)
    m2 = part[:, JT * 2 - 2].astype(np.float64)  # 2nd largest
    d2 = 2.0 - 2.0 * m2 / (SCALE * SCALE)
    loss = float(np.mean(-0.5 * np.log(d2)))
    return np.array(loss, dtype=np.float32)


_LAST_RESULTS = None  # BassKernelResults of the most recent run (for test.py)


def run(x: np.ndarray, trace: bool = False):
    global _LAST_RESULTS
    nc = build_bass()
    res = bass_utils.run_bass_kernel_spmd(
        nc,
        make_in_maps(x),
        core_ids=list(range(NCORES)),
        trace=trace,
        trace_cores=list(range(NCORES)) if trace else None,
    )
    _LAST_RESULTS = res
    return reduce_outputs(res.results)


def kernel(**inputs) -> np.ndarray:
    x = np.asarray(inputs["student_output"], dtype=np.float32)
    assert x.shape == (B, D), x.shape
    return run(x, trace=False)


if __name__ == "__main__":
    rng = np.random.default_rng(0)
    x = rng.standard_normal((B, D), dtype=np.float32)
    print(kernel(student_output=x))


# revision 9
# speedup vs baseline: 1.9676x; 1.9676x over previous
"""KoLeo loss kernel for Trainium2 (8 NeuronCores, Bass/Tile), fp8 edition.

reference semantics:
    x = student_output / max(||row||_2, 1e-8)        # [B, D] row-normalize
    dots = x @ x.T ; dots[i,i] = -1
    nn = argmax(dots, axis=1)
    d_i = || x_i - x_nn(i) + 1e-8 ||_2
    loss = mean(-log(d_i + 1e-8))

Device strategy (data-parallel over rows, 8 cores, identical NEFF):
  * Host pre-normalizes rows in fp32, scales by S=128, quantizes to
    fp8 e4m3 (TRN FP8_EXP4: max normal 240 > S) and ships the
    transposed layout [KT=8, 128, B].  Numpy-validated rel err 1.5e-4.
  * Gram tiles run as fp8 DoubleRow matmuls: each MM consumes TWO
    128-deep k-tiles ([128, 2, free] APs) at 2x bf16 PE throughput.
    Rows are unit-norm pre-quantization so no scaling pass is needed;
    the diagonal entry ~S^2 dominates, the global 2nd-max of a row is
    S^2 * the NN cosine.
  * Input DMAs alternate between both HWDGE queues (SP / ACT) at
    k-tile granularity, ordered stripe-0-first so the first DoubleRow
    matmul is gated by ~4 slices, not the whole 8 MB load.
  * Loop: jh-stripes of 2048 columns outer, mt (128-row chunk) inner;
    per (stripe, mt) group 4 psum tiles [128, 512] each accumulate 4
    DoubleRow MMs; DVE max8 drains PSUM directly into a candidate
    buffer.  4 psum tags x 2 bufs = all 8 PSUM banks ping-pong; dummy
    matmuls during the first DMA keep the PE/HAM clock warm.
  * Device ships the top-2 per (mt, j-tile) candidate values; the host
    takes the global 2nd max per row (top-1 is the row's self-dot),
    computes d^2 = 2 - 2 m~ / S^2 and the final mean of logs.
"""

import numpy as np
import ml_dtypes

import concourse.bacc as bacc
import concourse.bass as bass
import concourse.mybir as mybir
import concourse.tile as tile
from concourse import bass_utils

B, D, P = 8192, 1024, 128
NCORES = 8
LOCAL = B // NCORES  # 1024 rows per core
KT = D // P          # 8 contraction tiles
MT = LOCAL // P      # 8 local row tiles
NJ = 512             # moving free dim per matmul
JT = B // NJ         # 16 column tiles
JH = 4               # j-tiles per stripe
NSTRIPE = JT // JH   # 4 stripes of 2048 columns
SCALE = 128.0        # fp8 pre-scale; diag ~ S^2

F32 = mybir.dt.float32
FP8 = mybir.dt.float8e4
DR = mybir.MatmulPerfMode.DoubleRow


def emit_kernel(tc, x_ap, xl_ap, out_ap):
    nc = tc.nc
    with (
        tc.tile_pool(name="big", bufs=1) as big,
        tc.tile_pool(name="ps", bufs=2, space="PSUM") as pp,
    ):
        xT = big.tile([P, KT, B], FP8)
        xTl = big.tile([P, KT, LOCAL], FP8)
        cand = big.tile([P, MT, JT, 8], F32)
        warm = big.tile([P, NJ], FP8)

        nc.vector.memset(warm[:], 1.0)

        # --- loads: host ships fp8 already transposed as [KT, 128, B].
        # k-granular, stripe-0 first, alternating both HWDGE queues so the
        # first matmul group is gated by ~4 slices instead of the full 8 MB.
        for k in range(KT):
            nc.sync.dma_start(out=xTl[:, k], in_=xl_ap[k])
            cb = slice(0, JH * NJ)
            nc.scalar.dma_start(out=xT[:, k, cb], in_=x_ap[k, :, cb])
        for s in range(1, NSTRIPE):
            cb = slice(s * JH * NJ, (s + 1) * JH * NJ)
            for k in range(KT):
                q = nc.sync if k % 2 == 0 else nc.scalar
                q.dma_start(out=xT[:, k, cb], in_=x_ap[k, :, cb])

        # --- PE/HAM pre-warm: dummy matmuls on the memset tile while the
        # first stripe DMA is in flight (borrows one main-pool psum buffer;
        # finishes long before its rotation comes around again).
        wps = pp.tile([P, NJ], F32, tag="ps_u0", name="wps")
        for _ in range(12):
            nc.tensor.matmul(wps[:], warm[:, :P], warm[:], start=True, stop=True)

        # --- Gram: fp8 DoubleRow, 2 k-tiles per MM ------------------------
        for s in range(NSTRIPE):
            for mt in range(MT):
                ms = slice(mt * P, (mt + 1) * P)
                pss = [
                    pp.tile([P, NJ], F32, tag=f"ps_u{jj}", name=f"ps_u{jj}")
                    for jj in range(JH)
                ]
                for kk in range(KT // 2):
                    ks = slice(2 * kk, 2 * kk + 2)
                    for jj in range(JH):
                        j = s * JH + jj
                        nc.tensor.matmul(
                            pss[jj][:],
                            xTl[:, ks, ms],
                            xT[:, ks, j * NJ : (j + 1) * NJ],
                            start=(kk == 0),
                            stop=(kk == KT // 2 - 1),
                            perf_mode=DR,
                        )
                for jj in range(JH):
                    nc.vector.max(out=cand[:, mt, s * JH + jj], in_=pss[jj][:])

        # --- ship the whole candidate buffer (contiguous DMA); host picks
        # the global 2nd max per row and does the log/mean.
        nc.sync.dma_start(out=out_ap, in_=cand[:])


def build_bass():
    nc = bacc.Bacc(
        "TRN2",
        target_bir_lowering=False,
        debug=False,
        enable_asserts=True,
        num_devices=NCORES,
    )
    x_t = nc.dram_tensor("xq", [KT, P, B], FP8, kind="ExternalInput").ap()
    xl_t = nc.dram_tensor("xlq", [KT, P, LOCAL], FP8, kind="ExternalInput").ap()
    out_t = nc.dram_tensor("cand2", [P, MT, JT, 8], F32, kind="ExternalOutput").ap()
    with tile.TileContext(nc) as tc:
        emit_kernel(tc, x_t, xl_t, out_t)
    nc.compile()
    return nc


def make_in_maps(x: np.ndarray):
    norm = np.linalg.norm(x, axis=1, keepdims=True)
    xn = x / np.maximum(norm, 1e-8)
    q = (SCALE * xn).astype(ml_dtypes.float8_e4m3)
    # [KT, P, B]: element [k, p, r] = q[r, k*128 + p]  (transposed layout)
    xt = np.ascontiguousarray(q.reshape(B, KT, P).transpose(1, 2, 0))
    return [
        {
            "xq": xt,
            "xlq": np.ascontiguousarray(xt[:, :, c * LOCAL : (c + 1) * LOCAL]),
        }
        for c in range(NCORES)
    ]


def reduce_outputs(results):
    # cand2[p, mt, j, 0:2]: top-2 of S^2 * dots for local row mt*128+p over
    # column tile j.  Global top-1 is the self-dot (~S^2); the NN dot is the
    # global 2nd max.
    vals = np.concatenate(
        [
            r["cand2"].astype(np.float32)[:, :, :, 0:2].reshape(P, MT, JT * 2)
            for r in results
        ],
        axis=1,
    )  # [P, NCORES*MT, 32]
    vals = vals.transpose(1, 0, 2).reshape(B, JT * 2)  # row-major [B, 32]
    part = np.partition(vals, JT * 2 - 2, axis=1)
    m2 = part[:, JT * 2 - 2].astype(np.float64)  # 2nd largest
    d2 = 2.0 - 2.0 * m2 / (SCALE * SCALE)
    loss = float(np.mean(-0.5 * np.log(d2)))
    return np.array(loss, dtype=np.float32)


_LAST_RESULTS = None  # BassKernelResults of the most recent run (for test.py)


def run(x: np.ndarray, trace: bool = False):
    global _LAST_RESULTS
    nc = build_bass()
    res = bass_utils.run_bass_kernel_spmd(
        nc,
        make_in_maps(x),
        core_ids=list(range(NCORES)),
        trace=trace,
        trace_cores=list(range(NCORES)) if trace else None,
    )
    _LAST_RESULTS = res
    return reduce_outputs(res.results)


def kernel(**inputs) -> np.ndarray:
    x = np.asarray(inputs["student_output"], dtype=np.float32)
    assert x.shape == (B, D), x.shape
    return run(x, trace=False)


if __name__ == "__main__":
    rng = np.random.default_rng(0)
    x = rng.standard_normal((B, D), dtype=np.float32)
    print(kernel(student_output=x))


# revision 11
# speedup vs baseline: 2.7943x; 1.4202x over previous
"""KoLeo loss kernel for Trainium2 (8 NeuronCores, Bass/Tile).

fp8 DoubleRow + symmetric-Gram edition.

reference semantics:
    x = student_output / max(||row||_2, 1e-8)        # [B, D] row-normalize
    dots = x @ x.T ; dots[i,i] = -1
    nn = argmax(dots, axis=1)
    d_i = || x_i - x_nn(i) + 1e-8 ||_2
    loss = mean(-log(d_i + 1e-8))

Strategy:
  * Host pre-normalizes rows in fp32, scales by S=128, quantizes to fp8
    e4m3 (TRN FP8_EXP4 max normal 240 > S) and ships the transposed
    layout [KT=8, 128, B].  End-to-end numpy-validated rel err 1.4e-4.
  * dots is symmetric: only the upper triangle of the 16x16 grid of
    [512 x 512] blocks is computed -- 136 blocks, 17 per core.  All
    cores run the IDENTICAL block template
        {(0,0), (8,8), (0,8)} + {(0,d), (8,8+d) : d=1..7}
    over a column-strip ROTATED copy of x (core c's strip s = global
    strip (s+c) mod 16).  The 8 rotations tile all 136 blocks exactly
    once (verified), so the NEFF is the same for every core and only
    the input data differs.
  * Each [512x512] block: 16 fp8 DoubleRow matmuls (2 k-tiles per MM,
    2x bf16 PE throughput) into 4 psum tiles [128,512].  ACT drains
    each psum tile to a bf16 SBUF copy; DVE max8 takes per-row tile
    maxima (row side); for off-diagonal blocks GPSIMD reduces the 4
    bf16 copies elementwise to macc[128,512] (column side), which is
    DMA'd to DRAM.
  * Host combine: for each global row, its NN dot is the max over the
    16 candidate values it receives (row-side tile maxima where the
    row's strip is the block's row side; partition-maxima of macc
    where it is the column side; for diagonal blocks the top-1 is the
    row's self-dot ~S^2 and the top-2 value is the candidate).  Then
    d^2 = 2 - 2 m~ / S^2, loss = mean(-0.5 log d^2).
"""

import numpy as np
import ml_dtypes

import concourse.bacc as bacc
import concourse.bass as bass
import concourse.mybir as mybir
import concourse.tile as tile
from concourse import bass_utils

B, D, P = 8192, 1024, 128
NCORES = 8
KT = D // P          # 8 contraction tiles
GS = 512             # strip size (block edge, also moving free dim)
NS = B // GS         # 16 strips
MT4 = GS // P        # 4 row chunks per block
SCALE = 128.0        # fp8 pre-scale; self-dot ~ S^2

# 17 blocks per core, ordered so the needed strips arrive incrementally:
# (0,0),(0,1)..(0,8),(8,8),(8,9)..(8,15)
TEMPLATE = [(0, b) for b in range(9)] + [(8, b) for b in range(8, 16)]
NBLK = len(TEMPLATE)           # 17
OFF_SLOTS = [t for t, (a, b) in enumerate(TEMPLATE) if a != b]
NOFF = len(OFF_SLOTS)          # 15

F32 = mybir.dt.float32
BF16 = mybir.dt.bfloat16
FP8 = mybir.dt.float8e4
DR = mybir.MatmulPerfMode.DoubleRow


def emit_kernel(tc, x_ap, rowc_ap, macc_ap):
    nc = tc.nc
    with (
        tc.tile_pool(name="big", bufs=1) as big,
        tc.tile_pool(name="work", bufs=3) as work,
        tc.tile_pool(name="ps", bufs=2, space="PSUM") as pp,
    ):
        xT = big.tile([P, KT, B], FP8)
        rowc = big.tile([P, NBLK, MT4, 8], F32)
        warm = big.tile([P, GS], FP8)

        nc.vector.memset(warm[:], 1.0)

        # --- input DMA: k-granular, 2-strip chunks, incremental strip
        # order.  First two chunks split across both HWDGE queues so block
        # 0 can start early; the rest stream on SP ahead of consumption.
        for ch in range(NS // 2):
            cb = slice(ch * 2 * GS, (ch + 1) * 2 * GS)
            for k in range(KT):
                if ch < 2:
                    q = nc.sync if k % 2 == 0 else nc.scalar
                else:
                    q = nc.sync
                q.dma_start(out=xT[:, k, cb], in_=x_ap[k, :, cb])

        # --- PE/HAM pre-warm on the memset tile during the first DMAs.
        wps = pp.tile([P, GS], F32, tag="ps_m0", name="wps")
        for _ in range(12):
            nc.tensor.matmul(wps[:], warm[:, :P], warm[:], start=True, stop=True)

        # --- 17 symmetric blocks ------------------------------------------
        noff = 0
        for t, (a, b) in enumerate(TEMPLATE):
            pss = [
                pp.tile([P, GS], F32, tag=f"ps_m{mt}", name=f"ps_m{mt}")
                for mt in range(MT4)
            ]
            for kk in range(KT // 2):
                ks = slice(2 * kk, 2 * kk + 2)
                for mt in range(MT4):
                    nc.tensor.matmul(
                        pss[mt][:],
                        xT[:, ks, a * GS + mt * P : a * GS + (mt + 1) * P],
                        xT[:, ks, b * GS : (b + 1) * GS],
                        start=(kk == 0),
                        stop=(kk == KT // 2 - 1),
                        perf_mode=DR,
                    )
            cp = work.tile([P, MT4, GS], BF16, tag="cp", name="cp")
            for mt in range(MT4):
                nc.scalar.copy(cp[:, mt], pss[mt][:])
                nc.vector.max(out=rowc[:, t, mt], in_=cp[:, mt])
            if a != b:
                t01 = work.tile([P, GS], BF16, tag="t01", name="t01")
                macc = work.tile([P, GS], BF16, tag="macc", name="macc")
                nc.vector.tensor_max(t01[:], cp[:, 0], cp[:, 1])
                nc.vector.tensor_max(macc[:], cp[:, 2], cp[:, 3])
                nc.vector.tensor_max(macc[:], macc[:], t01[:])
                nc.scalar.dma_start(out=macc_ap[noff], in_=macc[:])
                noff += 1

        nc.sync.dma_start(out=rowc_ap, in_=rowc[:])


def build_bass():
    nc = bacc.Bacc(
        "TRN2",
        target_bir_lowering=False,
        debug=False,
        enable_asserts=True,
        num_devices=NCORES,
    )
    x_t = nc.dram_tensor("xq", [KT, P, B], FP8, kind="ExternalInput").ap()
    rowc_t = nc.dram_tensor(
        "rowc", [P, NBLK, MT4, 8], F32, kind="ExternalOutput"
    ).ap()
    macc_t = nc.dram_tensor("macc", [NOFF, P, GS], BF16, kind="ExternalOutput").ap()
    with tile.TileContext(nc) as tc:
        emit_kernel(tc, x_t, rowc_t, macc_t)
    nc.compile()
    return nc


def make_in_maps(x: np.ndarray):
    norm = np.linalg.norm(x, axis=1, keepdims=True)
    xn = x / np.maximum(norm, 1e-8)
    q = (SCALE * xn).astype(ml_dtypes.float8_e4m3)
    # [KT, P, B]: element [k, p, r] = q[r, k*128 + p]  (transposed layout)
    xt = np.ascontiguousarray(q.reshape(B, KT, P).transpose(1, 2, 0))
    # core c sees the row axis rotated by c*GS: its strip s = global (s+c)%16
    return [
        {"xq": np.ascontiguousarray(np.roll(xt, -c * GS, axis=2))}
        for c in range(NCORES)
    ]


def reduce_outputs(results):
    cand = np.full((B, 16), -np.inf, np.float32)
    nsrc = np.zeros(B, np.int32)

    def put(rows, vals):
        cand[rows, nsrc[rows]] = vals
        nsrc[rows] += 1

    for c in range(NCORES):
        rowc = results[c]["rowc"].astype(np.float32)  # [P, NBLK, MT4, 8]
        macc = results[c]["macc"].astype(np.float32)  # [NOFF, P, GS]
        noff = 0
        for t, (a, b) in enumerate(TEMPLATE):
            ga, gb = (a + c) % NS, (b + c) % NS
            for mt in range(MT4):
                rows = np.arange(ga * GS + mt * P, ga * GS + (mt + 1) * P)
                if ga == gb:
                    # top-1 is the row's self-dot; top-2 is the candidate
                    put(rows, rowc[:, t, mt, 1])
                else:
                    put(rows, rowc[:, t, mt, 0])
            if ga != gb:
                rows = np.arange(gb * GS, (gb + 1) * GS)
                put(rows, macc[noff].max(axis=0))
                noff += 1

    assert (nsrc == 16).all()
    m2 = cand.max(axis=1).astype(np.float64)
    d2 = 2.0 - 2.0 * m2 / (SCALE * SCALE)
    loss = float(np.mean(-0.5 * np.log(d2)))
    return np.array(loss, dtype=np.float32)


_LAST_RESULTS = None  # BassKernelResults of the most recent run (for test.py)


def run(x: np.ndarray, trace: bool = False):
    global _LAST_RESULTS
    nc = build_bass()
    res = bass_utils.run_bass_kernel_spmd(
        nc,
        make_in_maps(x),
        core_ids=list(range(NCORES)),
        trace=trace,
        trace_cores=list(range(NCORES)) if trace else None,
    )
    _LAST_RESULTS = res
    return reduce_outputs(res.results)


def kernel(**inputs) -> np.ndarray:
    x = np.asarray(inputs["student_output"], dtype=np.float32)
    assert x.shape == (B, D), x.shape
    return run(x, trace=False)


if __name__ == "__main__":
    rng = np.random.default_rng(0)
    x = rng.standard_normal((B, D), dtype=np.float32)
    print(kernel(student_output=x))
